# revision 2
# baseline (speedup 1.0000x reference)
# Trainium2 Bass kernel for nn_CBA (sparse attention style weighted reduction).
#
# reference:
#   prnt_lba[b,t] = lba_out[b, idx[b,t]]                       # gather rows
#   scores = concat([prnt_lba, embs], -1) @ W.sum(axis=1)      # [B, L]
#   w = exp(tanh(scores)); w /= (w.sum(-1) + EPS)
#   out[b] = sum_l w[b,l] * rnn_out[b,l]                       # [B, R]
#
# Key identity: the row gather followed by a dot with wsum[:R] equals a
# SCALAR gather of per-row dots:  s_lba[b,j] = lba_out[b,j,:] . wsum[:R]
# computed for all j, then scores[b,l] = s_lba[b, idx[b,l]] + embs dot.
# This turns a [B,L,R] gather into a [B,L] one and makes the kernel
# memory-bound: every big tensor is streamed exactly once.
#
# Sharding: data-parallel over batch, 4 batches per core, W replicated.

import os
import numpy as np
from contextlib import ExitStack

B, L, E, R = 32, 2048, 1024, 1024
NCORES = 8
BPC = B // NCORES          # batches per core
F = E + R                  # concat feature dim
EPS = 1e-7
NLT = L // 128             # l-tiles per batch (16)
CH = 2                     # l-tiles per DMA chunk

_PROG = None               # cached compiled Bass program
LAST_RESULTS = None        # BassKernelResults of the last run (for test.py)


def _build():
    import concourse.mybir as mybir
    import concourse.tile as tile
    from concourse import bacc, bass_isa
    from concourse.masks import make_identity

    f32 = mybir.dt.float32
    u16 = mybir.dt.uint16
    AOP = mybir.AluOpType
    AF = mybir.ActivationFunctionType

    nc = bacc.Bacc("TRN2", debug=False, enable_asserts=False,
                   target_bir_lowering=False, num_devices=NCORES)

    lba = nc.dram_tensor("lba", [BPC, L, R], f32, kind="ExternalInput").ap()
    emb = nc.dram_tensor("emb", [BPC, L, E], f32, kind="ExternalInput").ap()
    rnn = nc.dram_tensor("rnn", [BPC, L, R], f32, kind="ExternalInput").ap()
    wT = nc.dram_tensor("wT", [R, F], f32, kind="ExternalInput").ap()
    idxs = nc.dram_tensor("idxs", [BPC, 128, NLT], u16, kind="ExternalInput").ap()
    out = nc.dram_tensor("out", [BPC, R], f32, kind="ExternalOutput").ap()

    with tile.TileContext(nc) as tc, ExitStack() as ctx:
        cpool = ctx.enter_context(tc.tile_pool(name="const", bufs=1))
        identity = cpool.tile([128, 128], f32)
        make_identity(nc, identity)
        ones = cpool.tile([128, 1], f32)
        nc.vector.memset(ones, 1.0)
        # wsum[f] = sum_r W[f, r], replicated across all 128 partitions.
        wsum = cpool.tile([128, F], f32)
        with tc.tile_pool(name="wstage", bufs=1) as wpool:
            wst = wpool.tile([128, R // 128, F], f32)
            nc.sync.dma_start(wst, wT.rearrange("(a p) f -> p a f", p=128))
            wacc = wpool.tile([128, F], f32)
            nc.vector.tensor_reduce(wacc, wst.rearrange("p a f -> p f a"),
                                    axis=mybir.AxisListType.X, op=AOP.add)
            nc.gpsimd.partition_all_reduce(wsum, wacc, channels=128,
                                           reduce_op=bass_isa.ReduceOp.add)

        spool = ctx.enter_context(tc.tile_pool(name="streams", bufs=4))
        scratch = ctx.enter_context(tc.tile_pool(name="scratch", bufs=2))
        tabs = ctx.enter_context(tc.tile_pool(name="tabs", bufs=2))
        small = ctx.enter_context(tc.tile_pool(name="small", bufs=2))
        opool = ctx.enter_context(tc.tile_pool(name="outp", bufs=2))
        psmm = ctx.enter_context(tc.tile_pool(name="psmm", bufs=4, space="PSUM"))
        psden = ctx.enter_context(tc.tile_pool(name="psden", bufs=2, space="PSUM"))
        pstp = ctx.enter_context(tc.tile_pool(name="pstp", bufs=2, space="PSUM"))

        for b in range(BPC):
            # --- per-row dot products: s_lba[p, t] for l = 128*t + p ---
            s_lba = small.tile([128, NLT], f32, tag="slba")
            s_emb = small.tile([128, NLT], f32, tag="semb")
            for c in range(NLT // CH):
                rows = slice(c * CH * 128, (c + 1) * CH * 128)
                lt = spool.tile([128, CH, R], f32, tag="lba")
                nc.sync.dma_start(lt, lba[b, rows, :].rearrange("(a p) f -> p a f", p=128))
                et = spool.tile([128, CH, E], f32, tag="emb")
                nc.sync.dma_start(et, emb[b, rows, :].rearrange("(a p) f -> p a f", p=128))
                p1 = scratch.tile([128, CH, R], f32, tag="prod")
                for a in range(CH):
                    nc.vector.tensor_mul(p1[:, a, :], lt[:, a, :], wsum[:, 0:R])
                nc.vector.tensor_reduce(s_lba[:, c * CH:(c + 1) * CH], p1,
                                        axis=mybir.AxisListType.X, op=AOP.add)
                p2 = scratch.tile([128, CH, E], f32, tag="prod")
                for a in range(CH):
                    nc.vector.tensor_mul(p2[:, a, :], et[:, a, :], wsum[:, R:F])
                nc.vector.tensor_reduce(s_emb[:, c * CH:(c + 1) * CH], p2,
                                        axis=mybir.AxisListType.X, op=AOP.add)

            # --- gather s_lba[idx[l]] ---
            # flatten [128, 16] -> [1, 2048] in (p-major) table order:
            # table position p*NLT + t holds s_lba for l = 128*t + p.
            # host remaps indices j -> (j%128)*NLT + j//128 to match.
            flat = tabs.tile([1, L], f32, tag="flat")
            nc.scalar.dma_start(flat.rearrange("o (p t) -> o p t", p=128), s_lba)
            table = tabs.tile([128, L], f32, tag="table")
            nc.gpsimd.partition_broadcast(table, flat, channels=128)
            idxt = small.tile([128, NLT], u16, tag="idx")
            nc.scalar.dma_start(idxt, idxs[b])
            # G[16g + *, i] = s_lba[idx[b, 256g + i]]   (value for l = 256g + i)
            G = small.tile([128, 256], f32, tag="G")
            nc.gpsimd.indirect_copy(G, table, idxt, True)
            # transpose G back into (p, t) layout:
            #   l = 128t + p: t even -> T0[p, 8t], t odd -> T1[p, 8(t-1)]
            T0 = pstp.tile([128, 128], f32, tag="tp")
            nc.tensor.transpose(T0, G[:, 0:128], identity)
            T1 = pstp.tile([128, 128], f32, tag="tp")
            nc.tensor.transpose(T1, G[:, 128:256], identity)
            scl = small.tile([128, NLT], f32, tag="scl")
            scl3 = scl.rearrange("p (a two) -> p a two", two=2)
            nc.vector.tensor_copy(scl3[:, :, 0:1],
                                  T0.rearrange("p (a j) -> p a j", j=16)[:, :, 0:1])
            nc.vector.tensor_copy(scl3[:, :, 1:2],
                                  T1.rearrange("p (a j) -> p a j", j=16)[:, :, 0:1])

            # --- scores -> weights ---
            scores = small.tile([128, NLT], f32, tag="scores")
            nc.vector.tensor_add(scores, scl, s_emb)
            th = small.tile([128, NLT], f32, tag="th")
            nc.scalar.activation(th, scores, AF.Tanh)
            w = small.tile([128, NLT], f32, tag="w")
            nc.scalar.activation(w, th, AF.Exp)

            # --- out[b] = sum_l w[l] rnn[l, :] ; den = sum_l w[l] ---
            psA = psmm.tile([1, 512], f32, tag="mm")
            psB = psmm.tile([1, 512], f32, tag="mm")
            psD = psden.tile([1, 1], f32, tag="den")
            for c in range(NLT // CH):
                rows = slice(c * CH * 128, (c + 1) * CH * 128)
                rt = spool.tile([128, CH, R], f32, tag="rnn")
                nc.sync.dma_start(rt, rnn[b, rows, :].rearrange("(a p) f -> p a f", p=128))
                for a in range(CH):
                    t = c * CH + a
                    st, sp = (t == 0), (t == NLT - 1)
                    wcol = w[:, t:t + 1]
                    nc.tensor.matmul(psA, wcol, rt[:, a, 0:512], start=st, stop=sp)
                    nc.tensor.matmul(psB, wcol, rt[:, a, 512:1024], start=st, stop=sp)
                    nc.tensor.matmul(psD, wcol, ones, start=st, stop=sp)
            den = small.tile([1, 1], f32, tag="den_sb")
            nc.vector.tensor_scalar_add(den, psD, EPS)
            rinv = small.tile([1, 1], f32, tag="rinv")
            nc.vector.reciprocal(rinv, den)
            ot = opool.tile([1, R], f32, tag="ot")
            nc.scalar.activation(ot[:, 0:512], psA, AF.Copy, scale=rinv)
            nc.scalar.activation(ot[:, 512:1024], psB, AF.Copy, scale=rinv)
            nc.scalar.dma_start(out[b:b + 1, :], ot)

    nc.compile()
    return nc


def _get_prog():
    global _PROG
    if _PROG is None:
        _PROG = _build()
    return _PROG


def _marshal(embs, prnt_indices, lba_out, rnn_out, W):
    """Host-side input layout: shard over batch, transpose W, remap indices."""
    embs = np.ascontiguousarray(np.asarray(embs), dtype=np.float32)
    lba = np.ascontiguousarray(np.asarray(lba_out), dtype=np.float32)
    rnn = np.ascontiguousarray(np.asarray(rnn_out), dtype=np.float32)
    wT = np.ascontiguousarray(np.asarray(W, dtype=np.float32).T)
    idx = np.asarray(prnt_indices).astype(np.int64)

    # table position of original row j is (j % 128)*NLT + j // 128
    pos = ((idx % 128) * NLT + idx // 128).astype(np.uint16)  # [B, L]
    # wrap per gather group: group g covers l in [256g, 256g+256);
    # index i stored at partition 16g + i%16, column i//16
    A = pos.reshape(B, 8, 16, 16)            # [b, g, i//16, i%16]
    idxs_w = np.ascontiguousarray(A.transpose(0, 1, 3, 2).reshape(B, 128, NLT))

    in_maps = []
    for c in range(NCORES):
        s = slice(c * BPC, (c + 1) * BPC)
        in_maps.append({
            "lba": lba[s],
            "emb": embs[s],
            "rnn": rnn[s],
            "wT": wT,
            "idxs": idxs_w[s],
        })
    return in_maps


def kernel(embs, prnt_indices, lba_out, rnn_out, W):
    global LAST_RESULTS
    from concourse.bass_utils import run_bass_kernel_spmd

    nc = _get_prog()
    in_maps = _marshal(embs, prnt_indices, lba_out, rnn_out, W)
    res = run_bass_kernel_spmd(nc, in_maps, core_ids=list(range(NCORES)))
    LAST_RESULTS = res
    out = np.concatenate([r["out"] for r in res.results], axis=0)
    return out.astype(np.float32)
